# revision 9
# baseline (speedup 1.0000x reference)
"""Depth-to-space (pixel shuffle / DUC) kernel for Trainium2.

Full op: x[16, 1216, 32, 32] f32 -> out[16, 19, 304, 304] f32 where
  out[b, c, i*8+r1, j*8+r2] = x[b, c*64 + r1*8 + r2, i, j]
and out is zero-padded from 256x256 up to 304x304 (bottom/right).

Sharding: pure data-parallel over batch, 2 examples per core on 8 cores.

Per-core dataflow (per (b, c) image):
  1) 8 gather DMAs (one per r1) place data so SBUF partition
     p = i*4 + (r1 % 4) holds, at free offset (r1//4)*256, the 8 channels'
     32-float rows in [r2][j] order (contiguous 128B runs on the DRAM side).
  2) One DVE strided copy per image transposes [r2][j] -> [j][r2] within
     each partition's two halves, producing full 256-float output rows.
  3) Two store DMAs per image (one per r1//4 half) write 128 rows x 1KB
     contiguous runs each.
Zero padding relies on ExternalOutput buffers being pre-zeroed by the
runner (both the native path and the PJRT/axon path guarantee this).

Per-DMA-instruction descriptor-generation overhead (~0.6-1us serialized) is
the main cost at this size, so the DMA count per core is kept to
38 images * (8 loads + 2 stores) = 380, spread over both HWDGE rings.
"""

import sys

if "/opt/trn_rl_repo" not in sys.path:
    sys.path.insert(0, "/opt/trn_rl_repo")

import numpy as np

B, CLASSES, R, H, W, OUT = 16, 19, 8, 32, 32, 304
HR = H * R  # 256
N_CORES = 8
BPC = B // N_CORES  # batches per core = 2

_NC_CACHE = {}


def build_nc(
    bpc=BPC,
    classes=CLASSES,
    zero_init=False,
    num_devices=N_CORES,
    repeats=1,
    load_engines=("sync", "scalar"),
    store_engines=("scalar", "sync"),
    copy_engines=("vector",),
    bufs=4,
):
    import concourse.bacc as bacc
    import concourse.mybir as mybir
    from concourse.tile import TileContext

    f32 = mybir.dt.float32
    # Bacc (not plain Bass): its compile() legalizes multi-sem sync waits
    # that walrus otherwise rejects ("Too many sync wait commands").
    nc = bacc.Bacc(
        "TRN2", target_bir_lowering=False, debug=False, num_devices=num_devices
    )
    x = nc.declare_dram_parameter("x", [bpc, classes * R * R, H, W], f32, isOutput=False)
    out = nc.declare_dram_parameter("out", [bpc, classes, OUT, OUT], f32, isOutput=True)

    def eng(name):
        return getattr(nc, name)

    n_load = 0
    n_store = 0
    n_copy = 0
    with TileContext(nc) as tc:
        with (
            tc.tile_pool(name="raw", bufs=bufs) as raw_pool,
            tc.tile_pool(name="row", bufs=bufs) as row_pool,
        ):
          for _rep in range(repeats):
            for b in range(bpc):
                xb = x[b].rearrange("(c r1 r2) i j -> c r1 r2 i j", c=classes, r1=R, r2=R)
                for c in range(classes):
                    raw = raw_pool.tile([128, 2 * HR], f32)
                    row = row_pool.tile([128, 2 * HR], f32)
                    if zero_init:
                        # CoreSim's init-tracking can't merge strided
                        # writers; a full-tile writer first keeps it happy.
                        nc.gpsimd.memset(raw[:], 0.0)
                    # partition p = i*4 + (r1>>1) ; free = (r1&1)*256 + r2*32 + j
                    # => output row y = i*8 + r1 = 2*p + (r1&1)
                    raw_v = raw[:].rearrange(
                        "(i q) (e r2 j) -> i q e r2 j", q=4, e=2, r2=R
                    )
                    for r1 in range(R):
                        q, e = r1 >> 1, r1 & 1
                        # src dims (i, r2, j); 128B contiguous runs
                        src = xb[c, r1].transpose([1, 0, 2])
                        dst = raw_v[:, q, e]  # dims (i, r2, j)
                        eng(load_engines[n_load % len(load_engines)]).dma_start(
                            out=dst, in_=src
                        )
                        n_load += 1
                    # in-partition [r2][j] -> [j][r2] transpose (both halves)
                    s2 = raw[:].rearrange("p (e r2 j) -> p e j r2", e=2, r2=R)
                    d2 = row[:].rearrange("p (e j r2) -> p e j r2", e=2, r2=R)
                    getattr(nc, copy_engines[n_copy % len(copy_engines)]).tensor_copy(
                        d2, s2
                    )
                    n_copy += 1
                    # store half e: rows y = 2p + e; partition-contiguous src,
                    # dst row-stride 2*304, 1KB contiguous runs, 128 rows/DMA
                    for e in range(2):
                        eng(store_engines[n_store % len(store_engines)]).dma_start(
                            out=out[b, c, e : HR : 2, 0:HR],
                            in_=row[:, e * HR : (e + 1) * HR],
                        )
                        n_store += 1
    nc.compile()
    return nc


def _get_nc():
    key = "main"
    if key not in _NC_CACHE:
        _NC_CACHE[key] = build_nc()
    return _NC_CACHE[key]


def kernel(x: np.ndarray) -> np.ndarray:
    from concourse.bass_utils import run_bass_kernel_spmd

    x = np.ascontiguousarray(x, dtype=np.float32)
    assert x.shape == (B, CLASSES * R * R, H, W), x.shape
    nc = _get_nc()
    in_maps = [{"x": x[k * BPC : (k + 1) * BPC]} for k in range(N_CORES)]
    res = run_bass_kernel_spmd(nc, in_maps, list(range(N_CORES)))
    return np.concatenate([res.results[k]["out"] for k in range(N_CORES)], axis=0)


# revision 14
# speedup vs baseline: 147.0177x; 147.0177x over previous
"""Depth-to-space (pixel shuffle / DUC) kernel for Trainium2.

Full op: x[16, 1216, 32, 32] f32 -> out[16, 19, 304, 304] f32 where
  out[b, c, i*8+r1, j*8+r2] = x[b, c*64 + r1*8 + r2, i, j]
and out is zero-padded from 256x256 up to 304x304 (bottom/right).

Sharding: pure data-parallel over batch, 2 examples per core on 8 cores.

Per-core dataflow (per (b, c) image):
  1) 8 gather DMAs (one per r1) place data so SBUF partition
     p = i*4 + (r1 % 4) holds, at free offset (r1//4)*256, the 8 channels'
     32-float rows in [r2][j] order (contiguous 128B runs on the DRAM side).
  2) One DVE strided copy per image transposes [r2][j] -> [j][r2] within
     each partition's two halves, producing full 256-float output rows.
  3) Two store DMAs per image (one per r1//4 half) write 128 rows x 1KB
     contiguous runs each.
Zero padding relies on ExternalOutput buffers being pre-zeroed by the
runner (both the native path and the PJRT/axon path guarantee this).

Per-DMA-instruction descriptor-generation overhead (~0.6-1us serialized) is
the main cost at this size, so the DMA count per core is kept to
38 images * (8 loads + 2 stores) = 380, spread over both HWDGE rings.
"""

import sys

if "/opt/trn_rl_repo" not in sys.path:
    sys.path.insert(0, "/opt/trn_rl_repo")

import numpy as np

B, CLASSES, R, H, W, OUT = 16, 19, 8, 32, 32, 304
HR = H * R  # 256
N_CORES = 8
BPC = B // N_CORES  # batches per core = 2

_NC_CACHE = {}


def build_nc(
    bpc=BPC,
    classes=CLASSES,
    zero_init=False,
    num_devices=N_CORES,
    repeats=1,
    loop_repeats=1,
    load_engines=("sync", "scalar"),
    store_engines=("scalar", "sync"),
    copy_engines=("vector",),
    bufs=4,
):
    import concourse.bacc as bacc
    import concourse.mybir as mybir
    from concourse.tile import TileContext

    f32 = mybir.dt.float32
    # Bacc (not plain Bass): its compile() legalizes multi-sem sync waits
    # that walrus otherwise rejects ("Too many sync wait commands").
    nc = bacc.Bacc(
        "TRN2", target_bir_lowering=False, debug=False, num_devices=num_devices
    )
    x = nc.declare_dram_parameter("x", [bpc, classes * R * R, H, W], f32, isOutput=False)
    out = nc.declare_dram_parameter("out", [bpc, classes, OUT, OUT], f32, isOutput=True)

    def eng(name):
        return getattr(nc, name)

    n_load = 0
    n_store = 0
    n_copy = 0
    with TileContext(nc) as tc:
        with (
            tc.tile_pool(name="raw", bufs=bufs) as raw_pool,
            tc.tile_pool(name="row", bufs=bufs) as row_pool,
        ):
          def _body():
            nonlocal n_load, n_store, n_copy
            for b in range(bpc):
                xb = x[b].rearrange("(c r1 r2) i j -> c r1 r2 i j", c=classes, r1=R, r2=R)
                for c in range(classes):
                    raw = raw_pool.tile([128, 2 * HR], f32)
                    row = row_pool.tile([128, 2 * HR], f32)
                    if zero_init:
                        # CoreSim's init-tracking can't merge strided
                        # writers; a full-tile writer first keeps it happy.
                        nc.gpsimd.memset(raw[:], 0.0)
                    # partition p = i*4 + (r1>>1) ; free = (r1&1)*256 + r2*32 + j
                    # => output row y = i*8 + r1 = 2*p + (r1&1)
                    # For fixed q = r1>>1 the needed channels c*64 + q*16 + m
                    # (m = (r1&1)*8 + r2 = 0..15) are 16 consecutive DRAM
                    # channels, and their free offsets m*32 are uniformly
                    # strided -> one 3-dim DMA per q: dims (i, m, j).
                    raw_v = raw[:].rearrange("(i q) (m j) -> i q m j", q=4, m=16)
                    for q in range(4):
                        # src dims (i, m, j); 128B contiguous runs
                        src = xb[c, 2 * q : 2 * q + 2].rearrange(
                            "e r2 i j -> i (e r2) j"
                        )
                        dst = raw_v[:, q]  # dims (i, m, j)
                        eng(load_engines[n_load % len(load_engines)]).dma_start(
                            out=dst, in_=src
                        )
                        n_load += 1
                    # in-partition [r2][j] -> [j][r2] transpose (both halves)
                    s2 = raw[:].rearrange("p (e r2 j) -> p e j r2", e=2, r2=R)
                    d2 = row[:].rearrange("p (e j r2) -> p e j r2", e=2, r2=R)
                    getattr(nc, copy_engines[n_copy % len(copy_engines)]).tensor_copy(
                        d2, s2
                    )
                    n_copy += 1
                    # store: rows y = 2p + e; one DMA per image, dims
                    # (p, e, w) on both sides, 1KB contiguous runs, 256 rows
                    eng(store_engines[n_store % len(store_engines)]).dma_start(
                        out=out[b, c, 0:HR, 0:HR].rearrange("(p e) w -> p e w", e=2),
                        in_=row[:].rearrange("p (e w) -> p e w", e=2),
                    )
                    n_store += 1

          if loop_repeats > 1:
              # measurement-only: on-device loop to amortize dispatch noise
              with tc.For_i(0, loop_repeats, 1):
                  _body()
          else:
              for _rep in range(repeats):
                  _body()
    nc.compile()
    return nc


def _get_nc():
    key = "main"
    if key not in _NC_CACHE:
        _NC_CACHE[key] = build_nc()
    return _NC_CACHE[key]


def kernel(x: np.ndarray) -> np.ndarray:
    from concourse.bass_utils import run_bass_kernel_spmd

    x = np.ascontiguousarray(x, dtype=np.float32)
    assert x.shape == (B, CLASSES * R * R, H, W), x.shape
    nc = _get_nc()
    in_maps = [{"x": x[k * BPC : (k + 1) * BPC]} for k in range(N_CORES)]
    res = run_bass_kernel_spmd(nc, in_maps, list(range(N_CORES)))
    return np.concatenate([res.results[k]["out"] for k in range(N_CORES)], axis=0)
